# revision 3
# baseline (speedup 1.0000x reference)
"""Multi-head attention (B=2, Q=K=2048, H=16, D=V=64) on 8 Trainium2 cores.

Sharding: batch x heads. Core c handles batch b = c//4 and heads
[4*(c%4), 4*(c%4)+4) -- 4 (b,h) "pairs" per core, no cross-core comm.

Key optimization vs the naive version: the key_mask zeroes ~half the
keys, and masked keys contribute exactly 0 to both the softmax numerator
and denominator (exp*mask). So the host compacts K/V to the unmasked
keys only (padded to a multiple of 128; pad keys have V''=0 and a zero
denominator column, so they contribute exactly nothing). This halves
the score matrix and with it the TensorE and ActE work. The host also
pre-transposes and pre-casts Q/K/V'' to bf16, so the device does no
input conversion at all.

Device algorithm per (b,h) pair (flash-style, no max subtraction:
scores/8 ~ N(0,1), so exp() is far from fp32 overflow; the reference's
max subtraction cancels in the softmax ratio up to a vanishing
eps*exp(-max) term):

  for each q-block (512 wide):
    for each k-chunk (128 compacted keys):
      S^T[k,q] = (K-chunk d,k)^T @ (Q^T d,q)   on TensorE (bf16, fp32 acc)
      E = exp(S/8)                              on ScalarE, PSUM -> SBUF bf16
      acc[0:65, q] += V''^T @ E                 on TensorE (V'' = [V*m | m])
    normalize in O^T orientation: rec[q] = 1/(acc[64, q] + eps) on
    VectorE, partition-broadcast rec on GpSimd, then
    O^T[v, q] = acc[v, q] * rec[q] on VectorE. The host untransposes.

Software pipelining: QK matmuls for exp-group g are emitted before the
PV matmuls of group g-1, so the TensorE never waits on the ScalarE exp
(and vice versa). The normalize runs entirely on VectorE/GpSimd, so the
TensorE stream is matmuls only. PSUM: 2x3-bank score windows + 2 acc
banks = 8. Input DMAs are split across the SP and ActE queues so the
first pair's tiles land fast.
"""

import math
import sys

import numpy as np

sys.path.insert(0, "/opt/trn_rl_repo")

import ml_dtypes

import concourse.bacc as bacc
import concourse.mybir as mybir
import concourse.tile as tile
from concourse.bass_utils import run_bass_kernel_spmd

BF16NP = ml_dtypes.bfloat16

N_CORES = 8
B, Q, K, H, D, V = 2, 2048, 2048, 16, 64, 64
PAIRS = 4            # (b,h) pairs per core
QBW = 512            # q-block width
QB = Q // QBW        # 4 q-blocks
G = 3                # k-chunks per exp group (3 PSUM banks per window)
EPS = 1e-10

F32 = mybir.dt.float32
BF16 = mybir.dt.bfloat16

_cached = {}
LAST_RESULTS = None


def _build_program(kc):
    """kc = number of 128-key chunks after mask compaction."""
    nc = bacc.Bacc("TRN2", target_bir_lowering=False, debug=False, num_devices=N_CORES)

    qT = nc.dram_tensor("qT", [PAIRS, D, Q], BF16, kind="ExternalInput").ap()
    kT = nc.dram_tensor("kT", [PAIRS, D, kc * 128], BF16, kind="ExternalInput").ap()
    v = nc.dram_tensor("v", [PAIRS, 128, kc, V + 1], BF16, kind="ExternalInput").ap()
    # output: [pair, V, blk, q-in-block] (O^T; host untransposes)
    o = nc.dram_tensor("o", [PAIRS, V, QB, QBW], F32, kind="ExternalOutput").ap()

    with tile.TileContext(nc) as tc:
        with (
            tc.sbuf_pool(name="persist", bufs=1) as persist,
            tc.sbuf_pool(name="epool", bufs=3) as epool,
            tc.sbuf_pool(name="norm", bufs=2) as normp,
            tc.sbuf_pool(name="osbp", bufs=2) as osbp,
            tc.psum_pool(name="win", bufs=2) as winp,
            tc.psum_pool(name="acc", bufs=2) as accp,
        ):
            # ---------------- input DMAs (no device-side conversion) -------
            # pairs 0/1 stream in on the SP queue, pairs 2/3 on the ActE
            # queue, so pair 0's tiles are resident ASAP.
            qTb, kTb, vppb = [], [], []
            for p in range(PAIRS):
                eng = nc.sync if p < 2 else nc.scalar
                qb = persist.tile([D, Q], BF16, tag=f"qTb{p}")
                eng.dma_start(out=qb, in_=qT[p])
                qTb.append(qb)
                kb = persist.tile([D, kc * 128], BF16, tag=f"kTb{p}")
                eng.dma_start(out=kb, in_=kT[p])
                kTb.append(kb)
                vt = persist.tile([128, kc, V + 1], BF16, tag=f"vpp{p}")
                eng.dma_start(out=vt, in_=v[p])
                vppb.append(vt)

            groups = [list(range(s, min(s + G, kc))) for s in range(0, kc, G)]

            def emit_mm2(p, acc, chunks, e):
                for i, c in enumerate(chunks):
                    nc.tensor.matmul(
                        acc[:, :],
                        vppb[p][:, c, :],
                        e[:, i, :],
                        start=(c == 0),
                        stop=(c == kc - 1),
                    )

            def emit_norm(acc, blk, osb):
                den = normp.tile([1, QBW], F32, tag="den")
                nc.vector.tensor_scalar_add(out=den, in0=acc[V : V + 1, :], scalar1=EPS)
                rec = normp.tile([1, QBW], F32, tag="rec")
                nc.vector.reciprocal(out=rec, in_=den)
                bc = normp.tile([V, QBW], F32, tag="bc")
                nc.gpsimd.partition_broadcast(bc, rec)
                nc.vector.tensor_tensor(
                    out=osb[:, blk, :], in0=acc[0:V, :], in1=bc, op=mybir.AluOpType.mult
                )

            # ---------------- main pipelined loops ----------------
            for p in range(PAIRS):
                osb = osbp.tile([V, QB, QBW], F32, tag="osb")
                for blk in range(QB):
                    q0 = blk * QBW
                    acc = accp.tile([V + 1, QBW], F32, tag="acc")
                    pending = None  # (chunks, e) awaiting PV matmul
                    for gi, chunks in enumerate(groups):
                        win = winp.tile([128, G, QBW], F32, tag="win")
                        for i, c in enumerate(chunks):
                            nc.tensor.matmul(
                                win[:, i, :],
                                kTb[p][:, c * 128 : (c + 1) * 128],
                                qTb[p][:, q0 : q0 + QBW],
                                start=True,
                                stop=True,
                            )
                        if pending is not None:
                            emit_mm2(p, acc, *pending)
                        n = len(chunks)
                        e = epool.tile([128, G, QBW], BF16, tag="e")
                        nc.scalar.activation(
                            out=e[:, :n, :],
                            in_=win[:, :n, :],
                            func=mybir.ActivationFunctionType.Exp,
                            scale=0.125,
                        )
                        pending = (chunks, e)
                    emit_mm2(p, acc, *pending)
                    emit_norm(acc, blk, osb)
                nc.sync.dma_start(out=o[p], in_=osb)

    nc.compile()
    return nc


def _get_program(kc):
    if kc not in _cached:
        _cached[kc] = _build_program(kc)
    return _cached[kc]


def _shard_inputs(queries, keys, values, key_mask):
    q = np.asarray(queries, dtype=np.float32)
    k = np.asarray(keys, dtype=np.float32)
    v = np.asarray(values, dtype=np.float32)
    m = np.asarray(key_mask)

    idx = [np.nonzero(m[b])[0] for b in range(B)]
    keff = max(len(ix) for ix in idx)
    kc = max(1, math.ceil(keff / 128))
    kp = kc * 128

    # [B, S, H, D] -> [B, H, D, S], bf16
    qT = np.ascontiguousarray(q.transpose(0, 2, 3, 1)).astype(BF16NP)

    # compacted K^T and V'' = [V*m | m], zero-padded to kp keys
    kT = np.zeros((B, H, D, kp), dtype=np.float32)
    vpp = np.zeros((B, H, kp, V + 1), dtype=np.float32)
    for b in range(B):
        ix = idx[b]
        n = len(ix)
        if n == 0:
            continue
        mb = m[b, ix].astype(np.float32)
        kT[b, :, :, :n] = k[b, ix].transpose(1, 2, 0)
        vpp[b, :, :n, :V] = (v[b, ix] * mb[:, None, None]).transpose(1, 0, 2)
        vpp[b, :, :n, V] = mb[None, :]
    kTb = kT.astype(BF16NP)
    # [B, H, kp, V+1] -> [B, H, 128(r), kc, V+1]  (key kk = c*128 + r)
    vppb = np.ascontiguousarray(
        vpp.reshape(B, H, kc, 128, V + 1).transpose(0, 1, 3, 2, 4)
    ).astype(BF16NP)

    in_maps = []
    for core in range(N_CORES):
        b, h0 = core // 4, (core % 4) * 4
        in_maps.append(
            {
                "qT": np.ascontiguousarray(qT[b, h0 : h0 + 4]),
                "kT": np.ascontiguousarray(kTb[b, h0 : h0 + 4]),
                "v": np.ascontiguousarray(vppb[b, h0 : h0 + 4]),
            }
        )
    return in_maps, kc


def kernel(queries, keys, values, key_mask):
    global LAST_RESULTS
    in_maps, kc = _shard_inputs(queries, keys, values, key_mask)
    nc = _get_program(kc)
    res = run_bass_kernel_spmd(nc, in_maps, list(range(N_CORES)))
    LAST_RESULTS = res

    out = np.empty((B, Q, H * V), dtype=np.float32)
    for core in range(N_CORES):
        b, h0 = core // 4, (core % 4) * 4
        # [PAIRS, V, QB, QBW] -> [PAIRS, Q, V]
        oc = (
            res.results[core]["o"]
            .transpose(0, 2, 3, 1)
            .reshape(PAIRS, Q, V)
        )
        for p in range(PAIRS):
            h = h0 + p
            out[b, :, h * V : (h + 1) * V] = oc[p]
    return out
